# revision 23
# baseline (speedup 1.0000x reference)
"""ComplexAttentionBlock on 8 Trainium2 NeuronCores.

Sharding: 8 cores = (4 batches) x (2 query-token halves). Each core computes
LayerNorm+QKV for all 1024 tokens of its batch (k/v need the full sequence),
but attention scores / attention output / proj / LN2 / MLP only for its 512
query tokens.  Tokens are permuted on host so every core's query tokens are
rows 0:511 of its input -> fully symmetric SPMD program, no collectives.

Layouts on chip:
  - activations feeding matmuls are feature-major ("T" = [feat, tok]) so the
    contraction dim always sits on SBUF partitions; v is token-major for the
    attention*V matmul.  Weights are pre-transposed on host to [in, out].
  - all matmul operands are bf16 (PSUM accumulation stays f32); elementwise
    math (LN, softmax pieces, residual) stays f32.
  - q and k are built in a head-stacked complex layout [re_h; im_h] on the
    128 partitions via host-interleaved weights, so each attention score
    component needs ONE full-contraction matmul instead of two half-height
    ones:  sr = [kr;ki].[qr;qi],  si = [kr;ki].[-qi;qr].
  - v is kept in SBUF as [tok128, kt, head, re|im]; attn*V computes both
    components in one matmul (stationary [vr_h|vi_h]).  The attention
    output stays in SBUF (feature-major) and feeds proj directly.
  - softmax needs no max-subtraction (|attn|/8 is small so exp is safe); row
    sums come from a ones-column matmul.
"""

import sys

sys.path.insert(0, "/opt/trn_rl_repo")

import numpy as np

import concourse.bacc as bacc
import concourse.tile as tile
from concourse import mybir
from concourse.bass_utils import run_bass_kernel_spmd
from concourse.masks import make_identity

F32 = mybir.dt.float32
BF16 = mybir.dt.bfloat16
BF16_NP = mybir.dt.np(mybir.dt.bfloat16)
AF = mybir.ActivationFunctionType
OP = mybir.AluOpType

B, N, E, H, HD, MLP = 4, 1024, 768, 12, 64, 1536
NQ = N // 2          # query tokens per core
CE = E // 128        # 6 contraction chunks over E
CM = MLP // 128      # 12 chunks over MLP dim
EPS = 1e-6

NCORES = 8


def _mm(nc, ps, lhsT, rhs, start, stop):
    nc.tensor.matmul(ps, lhsT, rhs, start=start, stop=stop)


def build_program(iters=1):
    """Build the SPMD program. iters>1 repeats the whole body (identical
    work) -- used only by test.py's differential timing estimator."""
    nc = bacc.Bacc("TRN2", target_bir_lowering=False, debug=False)

    def din(name, shape, dt=BF16):
        return nc.dram_tensor(name, list(shape), dt, kind="ExternalInput").ap()

    xp = din("xp", [N, 2 * E], F32)
    q_wst = din("q_wst", [E, 2, H, 128])        # [re|im]-stacked q weights
    k_wst = din("k_wst", [E, 2, H, 128])        # [re|im]-stacked k weights
    # 4-plane (wr, wi, -wi, wr) weights: rhs [wr|wi] and [-wi|wr] slices let
    # one wide matmul produce both complex components per stationary load,
    # halving the PE-sequencer instruction count for these layers.
    v_wt4 = din("v_wt4", [E, 4, E])
    pj_wt4 = din("pj_wt4", [E, 4, E])
    m1_wt3 = din("m1_wt3", [E, 3, MLP])
    m2_wt4 = din("m2_wt4", [MLP, 4, E])
    qsb = din("qsb", [128, H], F32)             # stacked q bias [br_h; bi_h]
    ksb = din("ksb", [128, H], F32)
    m1_bcr = din("m1_bcr", [128, CM], F32)
    m1_bci = din("m1_bci", [128, CM], F32)
    pj_brr = din("pj_brr", [128, E], F32)
    pj_bri = din("pj_bri", [128, E], F32)
    m2_brr = din("m2_brr", [128, E], F32)
    m2_bri = din("m2_bri", [128, E], F32)
    g1r = din("g1r", [128, CE], F32)
    g1i = din("g1i", [128, CE], F32)
    b1r = din("b1r", [128, CE], F32)
    b1i = din("b1i", [128, CE], F32)
    g2r = din("g2r", [128, CE], F32)
    g2i = din("g2i", [128, CE], F32)
    b2r = din("b2r", [128, CE], F32)
    b2i = din("b2i", [128, CE], F32)
    ones_in = din("ones_in", [128, 1])
    qkv_vbr = din("qkv_vbr", [64, H], F32)
    qkv_vbi = din("qkv_vbi", [64, H], F32)

    out = nc.dram_tensor("out", [NQ, 2 * E], F32, kind="ExternalOutput").ap()

    with tile.TileContext(nc) as tc, tc.tile_pool(name="const", bufs=1) as const:
        ident = const.tile([128, 128], BF16)
        make_identity(nc, ident)
        ones_sb = const.tile([128, 1], BF16)
        nc.sync.dma_start(out=ones_sb, in_=ones_in)
        epsl = const.tile([128, 1], F32)
        nc.vector.memset(epsl, EPS)
        epsa = const.tile([128, 1], F32)
        nc.vector.memset(epsa, 1e-8)
        qsb_sb = const.tile([128, H], F32)
        nc.sync.dma_start(out=qsb_sb, in_=qsb)
        ksb_sb = const.tile([128, H], F32)
        nc.sync.dma_start(out=ksb_sb, in_=ksb)
        m1br_sb = const.tile([128, CM], F32)
        nc.sync.dma_start(out=m1br_sb, in_=m1_bcr)
        m1bi_sb = const.tile([128, CM], F32)
        nc.sync.dma_start(out=m1bi_sb, in_=m1_bci)
        vbr_sb = const.tile([64, H], F32)
        nc.sync.dma_start(out=vbr_sb, in_=qkv_vbr)
        vbi_sb = const.tile([64, H], F32)
        nc.sync.dma_start(out=vbi_sb, in_=qkv_vbi)
        gb_sb = {}
        for nm, src in (("g1r", g1r), ("g1i", g1i), ("b1r", b1r), ("b1i", b1i),
                        ("g2r", g2r), ("g2i", g2i), ("b2r", b2r), ("b2i", b2i)):
            t = const.tile([128, CE], F32, name=nm, tag=nm)
            nc.sync.dma_start(out=t, in_=src)
            gb_sb[nm] = t

        def layer_norm(lnp, psT, xr, xi, gcr, gci, bcr, bci, dst_r, dst_i,
                       dst_col, tok128):
            """One 128-token LN tile -> gamma/beta-folded feature-major dst."""
            sq = lnp.tile([128, E], F32, tag="sq")
            nc.gpsimd.tensor_mul(sq, xr, xr)
            sqi = lnp.tile([128, E], F32, tag="sqi")
            nc.scalar.square(sqi, xi)
            ssum = lnp.tile([128, 2], F32, tag="ssum")
            nc.vector.scalar_tensor_tensor(sq, in0=sq, scalar=0.0, in1=sqi,
                                           op0=OP.add, op1=OP.add,
                                           accum_out=ssum[:, 0:1])
            mag = lnp.tile([128, E], F32, tag="mag")
            nc.scalar.activation(mag, sq, AF.Sqrt, bias=epsl,
                                 accum_out=ssum[:, 1:2])
            # mean/var from the free accumulations: mean = sum(mag)/E,
            # var = sum(mag^2)/E + EPS - mean^2  (mag^2 = m2 + EPS exactly)
            mv = lnp.tile([128, 4], F32, tag="mv")
            nc.vector.tensor_scalar_mul(mv[:, 0:1], ssum[:, 1:2], 1.0 / E)
            nc.vector.tensor_scalar(out=mv[:, 2:3], in0=ssum[:, 0:1],
                                    scalar1=1.0 / E, scalar2=EPS,
                                    op0=OP.mult, op1=OP.add)
            nc.vector.tensor_scalar(out=mv[:, 3:4], in0=mv[:, 0:1],
                                    scalar1=mv[:, 0:1], scalar2=None,
                                    op0=OP.mult)
            nc.vector.scalar_tensor_tensor(mv[:, 1:2], in0=mv[:, 2:3],
                                           scalar=0.0, in1=mv[:, 3:4],
                                           op0=OP.add, op1=OP.subtract)
            rstd = lnp.tile([128, 1], F32, tag="rstd")
            nc.scalar.activation(rstd, mv[:, 1:2], AF.Sqrt, bias=epsl)
            nc.vector.reciprocal(rstd, rstd)
            d = lnp.tile([128, E], F32, name="d", tag="sq")
            nc.scalar.activation(d, mag, AF.Copy, bias=EPS)
            nc.vector.reciprocal_approx_fast(out=d, in_=d)
            scl = lnp.tile([128, E], F32, name="scl", tag="sqi")
            nc.vector.scalar_tensor_tensor(scl, in0=mag, scalar=mv[:, 0:1],
                                           in1=d, op0=OP.subtract, op1=OP.mult)
            x0r = lnp.tile([128, E], BF16, tag="x0r")
            nc.vector.scalar_tensor_tensor(x0r, in0=xr, scalar=rstd, in1=scl,
                                           op0=OP.mult, op1=OP.mult)
            x0i = lnp.tile([128, E], BF16, tag="x0i")
            nc.vector.scalar_tensor_tensor(x0i, in0=xi, scalar=rstd, in1=scl,
                                           op0=OP.mult, op1=OP.mult)
            for c in range(CE):
                for x0, dst, gc, bc, tg in ((x0r, dst_r, gcr, bcr, "pstr"),
                                            (x0i, dst_i, gci, bci, "psti")):
                    pst = psT.tile([128, 128], BF16, name=tg, tag=tg)
                    nc.tensor.transpose(pst, x0[:, c * 128:(c + 1) * 128], ident)
                    nc.vector.tensor_scalar(
                        out=dst[:, dst_col(c):dst_col(c) + tok128],
                        in0=pst, scalar1=gc[:, c:c + 1], scalar2=bc[:, c:c + 1],
                        op0=OP.mult, op1=OP.add)

        for _it in range(iters):
            # attn-scope tensors: v (token-major, head-interleaved re|im) and
            # the attention output (feature-major), both SBUF-resident.
            with tc.tile_pool(name="attn", bufs=1) as attn_p:
                v_sb = attn_p.tile([128, N // 128, H, 128], BF16)
                a_sb_r = attn_p.tile([128, CE * NQ], BF16)
                a_sb_i = attn_p.tile([128, CE * NQ], BF16)
                with tc.tile_pool(name="qk", bufs=1) as qk_p:
                    qs = qk_p.tile([128, H * NQ], BF16)
                    ks = qk_p.tile([128, H * N], BF16)

                    with tc.tile_pool(name="xnt", bufs=1) as xnt_p:
                        xnt_r = xnt_p.tile([128, CE * N], BF16)
                        xnt_i = xnt_p.tile([128, CE * N], BF16)

                        # q/k weight pools open early: their first weight DMAs
                        # prefetch during LN1/B.v instead of waiting for the
                        # previous pools' release.
                        with tc.tile_pool(name="wq", bufs=2) as wq, \
                             tc.tile_pool(name="wk", bufs=2) as wk, \
                             tc.tile_pool(name="wv", bufs=3) as wv:
                            # ------ stage A: LN1 + transpose, feature-major ------
                            with tc.tile_pool(name="lnA", bufs=2) as lnp, \
                                 tc.tile_pool(name="xin", bufs=4) as xin, \
                                 tc.tile_pool(name="psA", bufs=2, space="PSUM") as psA:
                                for t in range(N // 128):
                                    xt = xin.tile([128, 2 * E], F32, tag="xt")
                                    nc.sync.dma_start(out=xt, in_=xp[t * 128:(t + 1) * 128, :])
                                    xc = xt.rearrange("p (e c) -> p e c", c=2)
                                    layer_norm(lnp, psA, xc[:, :, 0], xc[:, :, 1],
                                               gb_sb["g1r"], gb_sb["g1i"], gb_sb["b1r"], gb_sb["b1i"],
                                               xnt_r, xnt_i, lambda c, t=t: c * N + t * 128, 128)

                            # ---- stage C pools open BEFORE B.q/B.k: the bump
                            # allocator adds release deps on reused zones, so
                            # opening these first lets attention overlap with
                            # the q/k matmul stages.
                            with tc.tile_pool(name="scp", bufs=2) as scp, \
                                 tc.tile_pool(name="rsp", bufs=2) as rsp, \
                                 tc.tile_pool(name="psS", bufs=2, space="PSUM") as psS:
                                # ------ stage B.q: head-stacked q ------
                                # weights stream 4 heads per DMA (256 KiB) so
                                # the per-DMA fixed cost keeps up with the PE.
                                with tc.tile_pool(name="psQ", bufs=4, space="PSUM") as psQ:
                                    for hg in range(3):
                                        ps = [psQ.tile([128, NQ], F32, name="qps", tag="qps")
                                              for _ in range(4)]
                                        for ci in range(CE):
                                            w4 = wq.tile([128, 2, 4, 128], BF16, tag="wq4")
                                            nc.sync.dma_start(out=w4, in_=q_wst[ci * 128:(ci + 1) * 128, :, 4 * hg:4 * hg + 4, :])
                                            xr_sl = xnt_r[:, ci * N:ci * N + NQ]
                                            xi_sl = xnt_i[:, ci * N:ci * N + NQ]
                                            for hh in range(4):
                                                _mm(nc, ps[hh], w4[:, 0, hh, :], xr_sl, ci == 0, False)
                                                _mm(nc, ps[hh], w4[:, 1, hh, :], xi_sl, False, ci == CE - 1)
                                        for hh in range(4):
                                            h = 4 * hg + hh
                                            nc.scalar.activation(qs[:, h * NQ:(h + 1) * NQ],
                                                                 ps[hh], AF.Identity,
                                                                 bias=qsb_sb[:, h:h + 1])

                                # ------ stage B.k: head-stacked k ------
                                with tc.tile_pool(name="psK", bufs=4, space="PSUM") as psK:
                                    for hg in range(6):
                                        ps = [[psK.tile([128, NQ], F32, name="kps", tag="kps")
                                               for _ in range(2)] for _ in range(2)]
                                        for ci in range(CE):
                                            w2 = wk.tile([128, 2, 2, 128], BF16, tag="wk2")
                                            nc.sync.dma_start(out=w2, in_=k_wst[ci * 128:(ci + 1) * 128, :, 2 * hg:2 * hg + 2, :])
                                            for tk in range(2):
                                                xr_sl = xnt_r[:, ci * N + tk * NQ:ci * N + (tk + 1) * NQ]
                                                xi_sl = xnt_i[:, ci * N + tk * NQ:ci * N + (tk + 1) * NQ]
                                                for hh in range(2):
                                                    _mm(nc, ps[hh][tk], w2[:, 0, hh, :], xr_sl, ci == 0, False)
                                                    _mm(nc, ps[hh][tk], w2[:, 1, hh, :], xi_sl, False, ci == CE - 1)
                                        for hh in range(2):
                                            h = 2 * hg + hh
                                            for tk in range(2):
                                                nc.vector.tensor_scalar_add(
                                                    ks[:, h * N + tk * NQ:h * N + (tk + 1) * NQ],
                                                    ps[hh][tk], ksb_sb[:, h:h + 1])

                                # ------ stage B.v: v -> SBUF (token-major) ------
                                # runs after B.k so its PE work overlaps the
                                # softmax elementwise wall of early attention.
                                with tc.tile_pool(name="psV", bufs=4, space="PSUM") as psV:
                                    for f in range(3):
                                        for tg in range(2):
                                            ps = [psV.tile([128, 512], F32, name="vps", tag="vps")
                                                  for _ in range(4)]
                                            for ci in range(CE):
                                                w4 = wv.tile([128, 4, 256], BF16, tag="wv3")
                                                nc.sync.dma_start(out=w4, in_=v_wt4[ci * 128:(ci + 1) * 128, :, f * 256:(f + 1) * 256])
                                                for t in range(4):
                                                    tok = tg * 4 + t
                                                    xr_sl = xnt_r[:, ci * N + tok * 128:ci * N + (tok + 1) * 128]
                                                    xi_sl = xnt_i[:, ci * N + tok * 128:ci * N + (tok + 1) * 128]
                                                    # one wide matmul per component pair:
                                                    # xr @ [wr|wi] then xi @ [-wi|wr]
                                                    _mm(nc, ps[t], xr_sl, w4[:, 0:2, :], ci == 0, False)
                                                    _mm(nc, ps[t], xi_sl, w4[:, 2:4, :], False, ci == CE - 1)
                                            for t in range(4):
                                                tok = tg * 4 + t
                                                # strided scatter: 4 heads' 64-col
                                                # slices -> [tok, head, re|im]
                                                dst_r = v_sb[:, tok, 4 * f:4 * f + 4, 0:64]
                                                dst_i = v_sb[:, tok, 4 * f:4 * f + 4, 64:128]
                                                src = ps[t].rearrange("p (c h d) -> p c h d", c=2, h=4)
                                                nc.vector.tensor_copy(dst_r, src[:, 0])
                                                nc.scalar.copy(dst_i, src[:, 1])

                                # ------ stage C: attention -> a_sb ------
                                # Head groups of G share ONE Sqrt + ONE Exp
                                # table load.  Per kt-tile both score
                                # components land in one 2-bank PSUM tile;
                                # sr egresses via DVE copy (f32->bf16, enables
                                # 2x-mode bf16 square), si egresses via ACT
                                # Square (square is in every ACT table).  The
                                # magnitude path runs in bf16 (validated:
                                # ~1e-3 extra out error vs 2e-2 budget).
                                GH = 3
                                NG = H // GH
                                with tc.tile_pool(name="aev", bufs=4) as aevp, \
                                     tc.tile_pool(name="m2p", bufs=1) as m2p, \
                                     tc.tile_pool(name="psAt", bufs=2, space="PSUM") as psAt, \
                                     tc.tile_pool(name="psSum", bufs=2, space="PSUM") as psSum:
                                    m2s = {}

                                    def emit_scores(g):
                                        """PE score matmuls + bf16 egress
                                        (DVE copy sr / ACT square si) + m2
                                        build (DVE bf16 square, Pool add)."""
                                        for hh in range(GH):
                                            h = g * GH + hh
                                            qs_h = qs[:, h * NQ:(h + 1) * NQ]
                                            qs2 = scp.tile([128, NQ], BF16, tag="qs2")
                                            nc.vector.tensor_scalar_mul(qs2[0:64, :], qs[64:128, h * NQ:(h + 1) * NQ], -1.0)
                                            nc.scalar.copy(qs2[64:128, :], qs[0:64, h * NQ:(h + 1) * NQ])
                                            srs = scp.tile([128, 8 * NQ], BF16, tag="srs", bufs=2)
                                            m2a = m2p.tile([128, 8 * NQ], BF16, tag="m2a",
                                                           bufs=2 * GH + 1)
                                            for kt in range(8):
                                                k_sl = ks[:, h * N + kt * 128:h * N + (kt + 1) * 128]
                                                sc = psS.tile([128, 2 * NQ], F32, tag="sc", bufs=2)
                                                _mm(nc, sc[:, 0:NQ], k_sl, qs_h, True, True)
                                                _mm(nc, sc[:, NQ:2 * NQ], k_sl, qs2, True, True)
                                                nc.vector.tensor_copy(srs[:, kt * NQ:(kt + 1) * NQ], sc[:, 0:NQ])
                                                nc.scalar.square(m2a[:, kt * NQ:(kt + 1) * NQ], sc[:, NQ:2 * NQ])
                                            for qt in range(4):
                                                # fine grain: m2 complete soon after
                                                # the last kt egress, so the ACT
                                                # sqrt/exp batch order survives the
                                                # scheduler (keeps table loads at
                                                # 2 per group).
                                                sl = slice(qt * 2 * NQ, (qt + 1) * 2 * NQ)
                                                sq2 = scp.tile([128, 2 * NQ], BF16, tag="sq2", bufs=2)
                                                nc.vector.tensor_mul(sq2, srs[:, sl], srs[:, sl])
                                                nc.gpsimd.tensor_add(m2a[:, sl], m2a[:, sl], sq2)
                                            m2s[h] = m2a

                                    def emit_magexp(g):
                                        """ACT sqrt batch then exp batch, in
                                        place -> one table load each."""
                                        for hh in range(GH):
                                            m2a = m2s[g * GH + hh]
                                            # sqrt(m2/64 + eps) = |s|/8
                                            nc.scalar.activation(m2a, m2a, AF.Sqrt,
                                                                 bias=epsa, scale=1.0 / 64.0)
                                        for hh in range(GH):
                                            m2a = m2s[g * GH + hh]
                                            nc.scalar.activation(m2a, m2a, AF.Exp)

                                    def emit_attnv(g):
                                        for hh in range(GH):
                                            h = g * GH + hh
                                            et = m2s.pop(h)
                                            ap = psAt.tile([128, NQ], F32, name="ap", tag="ap")
                                            sums = psSum.tile([1, NQ], F32, name="sums", tag="sums")
                                            for kt in range(8):
                                                et_sl = et[:, kt * NQ:(kt + 1) * NQ]
                                                first = kt == 0
                                                last = kt == N // 128 - 1
                                                _mm(nc, ap, v_sb[:, kt, h, :], et_sl, first, last)
                                                _mm(nc, sums, ones_sb, et_sl, first, last)
                                            rsum = rsp.tile([1, NQ], F32, tag="rsum")
                                            nc.vector.reciprocal_approx_fast(out=rsum, in_=sums)
                                            rsb = rsp.tile([128, NQ], F32, tag="rsb")
                                            nc.gpsimd.partition_broadcast(rsb, rsum)
                                            po = (h % 2) * 64
                                            csl = slice((h // 2) * NQ, (h // 2 + 1) * NQ)
                                            for comp, dst, bc in ((0, a_sb_r, vbr_sb), (1, a_sb_i, vbi_sb)):
                                                ev = aevp.tile([64, NQ], F32, name="aevt", tag="aevt")
                                                nc.vector.tensor_mul(ev, ap[comp * 64:comp * 64 + 64, :],
                                                                     rsb[comp * 64:comp * 64 + 64, :])
                                                nc.gpsimd.tensor_scalar_add(
                                                    dst[po:po + 64, csl], ev, bc[:, h:h + 1])

                                    # software pipeline: one-group score
                                    # lookahead keeps the PE warm while ACT
                                    # runs the sqrt/exp batches.
                                    emit_scores(0)
                                    emit_scores(1)
                                    emit_magexp(0)
                                    for g in range(2, NG):
                                        emit_scores(g)
                                        emit_attnv(g - 2)
                                        emit_magexp(g - 1)
                                    emit_attnv(NG - 2)
                                    emit_magexp(NG - 1)
                                    emit_attnv(NG - 1)

                # ------ stage D: proj + residual ------
                with tc.tile_pool(name="dbias", bufs=1) as dbias, \
                     tc.tile_pool(name="xc1", bufs=1) as xc1_p:
                    pjbr_sb = dbias.tile([128, E], F32)
                    nc.sync.dma_start(out=pjbr_sb, in_=pj_brr)
                    pjbi_sb = dbias.tile([128, E], F32)
                    nc.sync.dma_start(out=pjbi_sb, in_=pj_bri)
                    m2br_sb = dbias.tile([128, E], F32)
                    nc.sync.dma_start(out=m2br_sb, in_=m2_brr)
                    m2bi_sb = dbias.tile([128, E], F32)
                    nc.sync.dma_start(out=m2bi_sb, in_=m2_bri)
                    xc1_r = xc1_p.tile([128, (NQ // 128) * E], F32)
                    xc1_i = xc1_p.tile([128, (NQ // 128) * E], F32)
                    with tc.tile_pool(name="pw", bufs=2) as pw, \
                         tc.tile_pool(name="xqp", bufs=2) as xqp, \
                         tc.tile_pool(name="xbp", bufs=1) as xbp, \
                         tc.tile_pool(name="psD", bufs=8, space="PSUM") as psD:
                        xbr_all = xbp.tile([128, (NQ // 128) * E], F32)
                        xbi_all = xbp.tile([128, (NQ // 128) * E], F32)
                        for t in range(NQ // 128):
                            xq_t = xqp.tile([128, 2 * E], F32, tag="xq")
                            nc.sync.dma_start(out=xq_t, in_=xp[t * 128:(t + 1) * 128, :])
                            xqc = xq_t.rearrange("p (e c) -> p e c", c=2)
                            nc.gpsimd.tensor_add(xbr_all[:, t * E:(t + 1) * E], xqc[:, :, 0], pjbr_sb)
                            nc.gpsimd.tensor_add(xbi_all[:, t * E:(t + 1) * E], xqc[:, :, 1], pjbi_sb)
                        for fc in range(3):
                            f0, fw = fc * 256, 256
                            # merged complex matmul: one [128,512] (single
                            # PSUM bank) out per stationary load -- halves
                            # the PE instruction count.
                            ps = [psD.tile([128, 2 * fw], F32, name="pjps", tag="pjps")
                                  for _ in range(4)]
                            for ci in range(CE):
                                w4 = pw.tile([128, 4, fw], BF16, tag="pw3")
                                nc.sync.dma_start(out=w4, in_=pj_wt4[ci * 128:(ci + 1) * 128, :, f0:f0 + fw])
                                for t in range(4):
                                    lhr = a_sb_r[:, ci * NQ + t * 128:ci * NQ + (t + 1) * 128]
                                    lhi = a_sb_i[:, ci * NQ + t * 128:ci * NQ + (t + 1) * 128]
                                    _mm(nc, ps[t], lhr, w4[:, 0:2, :], ci == 0, False)
                                    _mm(nc, ps[t], lhi, w4[:, 2:4, :], False, ci == CE - 1)
                            for t in range(4):
                                nc.vector.tensor_add(xc1_r[:, t * E + f0:t * E + f0 + fw],
                                                     ps[t][:, 0:fw], xbr_all[:, t * E + f0:t * E + f0 + fw])
                                nc.vector.tensor_add(xc1_i[:, t * E + f0:t * E + f0 + fw],
                                                     ps[t][:, fw:2 * fw], xbi_all[:, t * E + f0:t * E + f0 + fw])

                    # ------ stage E: LN2 + transpose ------
                    with tc.tile_pool(name="xn2", bufs=1) as xn2_p:
                        xn2t_r = xn2_p.tile([128, CE * NQ], BF16)
                        xn2t_i = xn2_p.tile([128, CE * NQ], BF16)
                        # mlp weight pools open early: weight DMAs prefetch
                        # during LN2 instead of gating on its pool release.
                        with tc.tile_pool(name="wm1", bufs=3) as wm1, \
                             tc.tile_pool(name="wm2", bufs=3) as wm2:
                            # ht + psM1 open before LN2's pools so the first MLP
                            # matmuls (token-half split) start mid-LN2.
                            with tc.tile_pool(name="ht", bufs=1) as ht_p:
                                hT_r = ht_p.tile([128, CM * NQ], BF16)
                                hT_i = ht_p.tile([128, CM * NQ], BF16)
                                with tc.tile_pool(name="psM1", bufs=6, space="PSUM") as psM1:
                                    with tc.tile_pool(name="lnE", bufs=3) as lnE, \
                                         tc.tile_pool(name="psE", bufs=1, space="PSUM") as psE:
                                        for t in range(NQ // 128):
                                            layer_norm(lnE, psE,
                                                       xc1_r[:, t * E:(t + 1) * E], xc1_i[:, t * E:(t + 1) * E],
                                                       gb_sb["g2r"], gb_sb["g2i"], gb_sb["b2r"], gb_sb["b2i"],
                                                       xn2t_r, xn2t_i, lambda c, t=t: c * NQ + t * 128, 128)

                                    # ------ stage F.1: MLP in ------
                                    for g in range(4):
                                        ps = [[psM1.tile([128, NQ], F32, name="m1ps", tag="m1ps")
                                               for _ in range(2)] for _ in range(3)]
                                        for ci in range(CE):
                                            w3 = wm1.tile([128, 3, 384], BF16, tag="m1w3")
                                            nc.sync.dma_start(out=w3, in_=m1_wt3[ci * 128:(ci + 1) * 128, :, g * 384:(g + 1) * 384])
                                            wr, wi, win = w3[:, 0, :], w3[:, 1, :], w3[:, 2, :]
                                            xr_sl = xn2t_r[:, ci * NQ:(ci + 1) * NQ]
                                            xi_sl = xn2t_i[:, ci * NQ:(ci + 1) * NQ]
                                            for j in range(3):
                                                wsl = slice(j * 128, (j + 1) * 128)
                                                _mm(nc, ps[j][0], wr[:, wsl], xr_sl, ci == 0, False)
                                                _mm(nc, ps[j][0], win[:, wsl], xi_sl, False, ci == CE - 1)
                                                _mm(nc, ps[j][1], wi[:, wsl], xr_sl, ci == 0, False)
                                                _mm(nc, ps[j][1], wr[:, wsl], xi_sl, False, ci == CE - 1)
                                        for j in range(3):
                                            co = 3 * g + j
                                            nc.scalar.activation(hT_r[:, co * NQ:(co + 1) * NQ], ps[j][0],
                                                                 AF.Gelu, bias=m1br_sb[:, co:co + 1])
                                            nc.scalar.activation(hT_i[:, co * NQ:(co + 1) * NQ], ps[j][1],
                                                                 AF.Gelu, bias=m1bi_sb[:, co:co + 1])

                                with tc.tile_pool(name="outp", bufs=1) as outp, \
                                     tc.tile_pool(name="fp", bufs=2) as fp, \
                                     tc.tile_pool(name="psM2", bufs=8, space="PSUM") as psM2:
                                    ot = [outp.tile([128, 2 * E], F32, name=f"ot{t}", tag=f"ot{t}")
                                          for t in range(4)]
                                    for fc in range(3):
                                        f0, fw = fc * 256, 256
                                        ps = [psM2.tile([128, 2 * fw], F32, name="m2ps", tag="m2ps")
                                              for _ in range(4)]
                                        for ck in range(CM):
                                            w4 = wm2.tile([128, 4, fw], BF16, tag="m2w3")
                                            nc.sync.dma_start(out=w4, in_=m2_wt4[ck * 128:(ck + 1) * 128, :, f0:f0 + fw])
                                            for t in range(4):
                                                hr_sl = hT_r[:, ck * NQ + t * 128:ck * NQ + (t + 1) * 128]
                                                hi_sl = hT_i[:, ck * NQ + t * 128:ck * NQ + (t + 1) * 128]
                                                _mm(nc, ps[t], hr_sl, w4[:, 0:2, :], ck == 0, False)
                                                _mm(nc, ps[t], hi_sl, w4[:, 2:4, :], False, ck == CM - 1)
                                        for t in range(4):
                                            xcb_r = fp.tile([128, fw], F32, tag="xcbr")
                                            nc.vector.tensor_add(xcb_r, xc1_r[:, t * E + f0:t * E + f0 + fw],
                                                                 m2br_sb[:, f0:f0 + fw])
                                            xcb_i = fp.tile([128, fw], F32, tag="xcbi")
                                            nc.vector.tensor_add(xcb_i, xc1_i[:, t * E + f0:t * E + f0 + fw],
                                                                 m2bi_sb[:, f0:f0 + fw])
                                            oc = ot[t].rearrange("p (e c) -> p e c", c=2)
                                            nc.vector.tensor_add(oc[:, f0:f0 + fw, 0], ps[t][:, 0:fw], xcb_r)
                                            nc.vector.tensor_add(oc[:, f0:f0 + fw, 1], ps[t][:, fw:2 * fw], xcb_i)
                                            if fc == 2:
                                                # tile complete after the last f chunk
                                                nc.sync.dma_start(out=out[t * 128:(t + 1) * 128, :], in_=ot[t])

    nc.compile()
    return nc


_NC = None


def _get_program():
    global _NC
    if _NC is None:
        _NC = build_program()
    return _NC


def make_in_maps(inputs):
    f = lambda a: np.ascontiguousarray(np.asarray(a, dtype=np.float32))
    bf = lambda a: np.ascontiguousarray(np.asarray(a).astype(BF16_NP))
    x = f(inputs["x"])
    g1, b1 = f(inputs["g1"]), f(inputs["b1"])
    g2, b2 = f(inputs["g2"]), f(inputs["b2"])

    common = {}
    qwrT = f(inputs["qkv_wr"]).T            # [E, 3E] (in, out)
    qwiT = f(inputs["qkv_wi"]).T

    def stack_qk(lo):
        A = np.empty((E, H, 128), np.float32)
        Bp = np.empty((E, H, 128), np.float32)
        for h in range(H):
            cr = slice(lo + h * 64, lo + (h + 1) * 64)
            A[:, h, 0:64] = qwrT[:, cr]
            A[:, h, 64:128] = qwiT[:, cr]
            Bp[:, h, 0:64] = -qwiT[:, cr]
            Bp[:, h, 64:128] = qwrT[:, cr]
        return bf(np.stack([A, Bp], axis=1))    # [E, 2, H, 128]

    common["q_wst"] = stack_qk(0)
    common["k_wst"] = stack_qk(E)
    # 4-plane (wr, wi, -wi, wr) layouts for the merged complex matmuls
    vwr, vwi = qwrT[:, 2 * E:], qwiT[:, 2 * E:]
    common["v_wt4"] = bf(np.stack([vwr, vwi, -vwi, vwr], axis=1))
    for nm, key in (("m2", "m2"), ("proj", "pj")):
        wr = f(inputs[f"{nm}_wr"]).T
        wi = f(inputs[f"{nm}_wi"]).T
        common[f"{key}_wt4"] = bf(np.stack([wr, wi, -wi, wr], axis=1))
    m1wr = f(inputs["m1_wr"]).T
    m1wi = f(inputs["m1_wi"]).T
    common["m1_wt3"] = bf(np.stack([m1wr, m1wi, -m1wi], axis=1))
    qbr, qbi = f(inputs["qkv_br"]), f(inputs["qkv_bi"])
    qsb = np.empty((128, H), np.float32)
    ksb = np.empty((128, H), np.float32)
    for h in range(H):
        qsb[0:64, h] = qbr[h * 64:(h + 1) * 64]
        qsb[64:128, h] = qbi[h * 64:(h + 1) * 64]
        ksb[0:64, h] = qbr[E + h * 64:E + (h + 1) * 64]
        ksb[64:128, h] = qbi[E + h * 64:E + (h + 1) * 64]
    common["qsb"] = qsb
    common["ksb"] = ksb
    m1br, m1bi = f(inputs["m1_br"]), f(inputs["m1_bi"])
    common["m1_bcr"] = np.ascontiguousarray(m1br.reshape(CM, 128).T)
    common["m1_bci"] = np.ascontiguousarray(m1bi.reshape(CM, 128).T)
    common["pj_brr"] = np.ascontiguousarray(np.tile(f(inputs["proj_br"])[None, :], (128, 1)))
    common["pj_bri"] = np.ascontiguousarray(np.tile(f(inputs["proj_bi"])[None, :], (128, 1)))
    common["m2_brr"] = np.ascontiguousarray(np.tile(f(inputs["m2_br"])[None, :], (128, 1)))
    common["m2_bri"] = np.ascontiguousarray(np.tile(f(inputs["m2_bi"])[None, :], (128, 1)))
    for nm, arr in (("g1", g1), ("b1", b1), ("g2", g2), ("b2", b2)):
        common[f"{nm}r"] = np.ascontiguousarray(arr[:, 0].reshape(CE, 128).T)
        common[f"{nm}i"] = np.ascontiguousarray(arr[:, 1].reshape(CE, 128).T)
    common["ones_in"] = np.ones((128, 1), BF16_NP)
    common["qkv_vbr"] = np.ascontiguousarray(qbr[2 * E:].reshape(H, 64).T)
    common["qkv_vbi"] = np.ascontiguousarray(qbi[2 * E:].reshape(H, 64).T)

    in_maps = []
    for core in range(NCORES):
        b, half = core // 2, core % 2
        if half == 0:
            xpm = x[b]
        else:
            xpm = np.concatenate([x[b, NQ:], x[b, :NQ]], axis=0)
        in_maps.append({"xp": np.ascontiguousarray(xpm), **common})
    return in_maps


def kernel(**inputs) -> np.ndarray:
    nc = _get_program()
    in_maps = make_in_maps(inputs)
    res = run_bass_kernel_spmd(nc, in_maps, list(range(NCORES)))
    out = np.empty((B, N, 2 * E), np.float32)
    for core in range(NCORES):
        b, half = core // 2, core % 2
        out[b, half * NQ:(half + 1) * NQ, :] = res.results[core]["out"]
    return out

